# revision 39
# baseline (speedup 1.0000x reference)
"""Trainium2 Bass kernel for nn_ContrastiveDist (supervised contrastive loss).

Math
----
The (n,n) distance/weight loss collapses to per-class statistics.  With
classes c = 0..15, cnt[c], class feature sums C[c,:], squared-norm sums
SqSum[c], global sums Ftot / SSall:

    alpha[c] = 1/(cnt[c]-1+eps),  beta[c] = 1/(n-cnt[c]+eps)
    P[c]   = alpha*cnt - beta*(n-cnt)
    Q[c]   = alpha*SqSum - beta*(SSall-SqSum)
    R[c,:] = 2*beta*(Ftot-C) - 2*alpha*C
    loss_i = f_i . R[c_i] + sq_i*P[c_i] + Q[c_i] + M
    result = sum(relu(loss_i)*valid_i) / max(sum(valid_i), 1)

Device pipeline (per core, replicated on all 8 cores; collectives in this
dispatch path cost ~70us extra so every core redundantly computes the
full loss):
  1. DMA features bf16 in two layouts on one HWDGE FIFO (row-tiled [F|1]
     first, small leading chunk so stats can start early; then d-major
     F^T).  One-hots ride the SWDGE queue (row-tiled early, class-group
     late).
  2. Squares chunk-wise, alternating ACT/DVE.
  3. Stats: per row tile, two contiguous FD-129 matmuls sharing one
     weight load accumulate eoh^T @ [F|1] and eoh^T @ [F^2|1] into a
     (16,258) PSUM bank. A zero-matmul pre-clears the bank so every real
     matmul runs start=False (start=True clears has_written bits for the
     WHOLE bank, which would corrupt the sibling region).
  4. Loss matrix L[c,i] = F^T.R^T + (F^T)^2.P_bcast into four 1-bank
     PSUM tiles (bank h = columns 512h..512h+512 of the row space), each
     laid out as 4 row-groups x 32 classes (classes duplicated into both
     16-halves to keep all PSUM elements written).
  5. Mask, per 512-column bank as soon as its A-chain lands: ACT computes
     relu(L + (Q+M)_percls) (bias = per-partition AP) in bf16, DVE
     multiplies by the class-group one-hot with a free-axis accumulate;
     final partition reduce via ones-matmul; divide; DMA the scalar out.

valid_i folds into the coefficients (R,P,(Q+M) zeroed for cnt<2).
Validated ~4.7e-5 rel vs fp64.
"""

import numpy as np
import ml_dtypes

import concourse.bacc as bacc
import concourse.tile as tile
import concourse.mybir as mybir
from concourse.bass_utils import run_bass_kernel_spmd

N, D, K, NCORES = 8192, 128, 16, 8
T = N // 128               # 64 row tiles of 128
W = D + 1                  # 129: [F | 1]
NG, GI = 4, 2048           # 4 row-groups of 2048 rows (loss layout)
EPS, MARGIN = 1e-6, 10.0
F32 = mybir.dt.float32
BF16 = mybir.dt.bfloat16
Alu = mybir.AluOpType
Act = mybir.ActivationFunctionType
AxX = mybir.AxisListType.X
BF = ml_dtypes.bfloat16

FRB = [0, 4, 16, 32, 48, 64]   # frd chunk boundaries (row tiles)

_CACHE: dict = {}


def _build():
    if "nc" in _CACHE:
        return _CACHE["nc"]

    nc = bacc.Bacc("TRN2", target_bir_lowering=False, debug=False,
                   num_devices=NCORES)
    frd = nc.dram_tensor("frd", [128, T * W], BF16, kind="ExternalInput").ap()
    ftd = nc.dram_tensor("ftd", [128, N], BF16, kind="ExternalInput").ap()
    eohb = nc.dram_tensor("eohb", [128, T * 16], BF16, kind="ExternalInput").ap()
    eohc = nc.dram_tensor("eohc", [128, GI], BF16, kind="ExternalInput").ap()
    csta = nc.dram_tensor("csta", [16, 272], F32, kind="ExternalInput").ap()
    cstb = nc.dram_tensor("cstb", [128, 1], F32, kind="ExternalInput").ap()
    res = nc.dram_tensor("res", [1, 1], F32, kind="ExternalOutput").ap()

    with tile.TileContext(nc) as tc:
        with (
            tc.tile_pool(name="sb", bufs=1) as sb,
            tc.tile_pool(name="ps", bufs=1, space="PSUM") as ps,
        ):
            # zero weights (PSUM bank clears + HAM pacing) -- memset first
            # so it isn't queued behind the gpsimd DMA dispatches
            zbig = sb.tile([128, 512], BF16)
            nc.gpsimd.memset(zbig[:], 0.0)

            # ---------------- loads ----------------
            cstas = sb.tile([16, 272], F32)
            nc.gpsimd.dma_start(cstas[:], csta)
            cstbs = sb.tile([128, 1], F32)
            nc.gpsimd.dma_start(cstbs[:], cstb)
            ssel = cstas[:, 0:128]      # Qcg scatter matrix
            idf = cstas[:, 128:144]     # identity 16x16
            ones16 = cstas[:, 144:160]  # ones (16,16)
            onesw = cstas[:, 144:272]   # ones (16,128)

            eohbs = sb.tile([128, T * 16], BF16)

            frgr = sb.tile([128, 2 * T * W], BF16)   # [F|1] rows then [F^2|1]
            fts = sb.tile([128, N], BF16)
            gts = sb.tile([128, N], BF16)
            eohcs = sb.tile([128, GI], BF16)
            NCT = 8
            TCH = N // NCT       # 1024 (2048B runs)
            # one HWDGE FIFO for all features: the row one-hot first (it
            # gates the first stats matmul), then frd (small leading chunk
            # so the stats stream starts early), then ftd
            nc.sync.dma_start(eohbs[:], eohb)
            for k in range(len(FRB) - 1):
                nc.sync.dma_start(frgr[:, FRB[k] * W:FRB[k + 1] * W],
                                  frd[:, FRB[k] * W:FRB[k + 1] * W])
            for k in range(NCT):
                nc.sync.dma_start(fts[:, k * TCH:(k + 1) * TCH],
                                  ftd[:, k * TCH:(k + 1) * TCH])
            # class-group one-hot: needed only at mask time, keep it last
            nc.gpsimd.dma_start(eohcs[:], eohc)

            # ------- squares, chunk-wise, alternating ACT/DVE -------
            # (ACT Square has no 16-bit accel ~2ns/elem; DVE TT bf16 runs
            # 2x but pays a DRAIN ~= dur after each op)
            for k in range(len(FRB) - 1):
                src = frgr[:, FRB[k] * W:FRB[k + 1] * W]
                dst = frgr[:, (T + FRB[k]) * W:(T + FRB[k + 1]) * W]
                if k % 2 == 0:
                    nc.scalar.activation(dst, src, Act.Square)
                else:
                    nc.vector.tensor_tensor(dst, src, src, op=Alu.mult)
            for k in range(NCT):
                src = fts[:, k * TCH:(k + 1) * TCH]
                dst = gts[:, k * TCH:(k + 1) * TCH]
                if k % 2 == 0:
                    nc.vector.tensor_tensor(dst, src, src, op=Alu.mult)
                else:
                    nc.scalar.activation(dst, src, Act.Square)

            # ---------------- stats: eoh^T @ [F|1|F^2|1] ----------------
            finP = ps.tile([1, 512], F32)
            eohb3 = eohbs.rearrange("p (t c) -> p t c", c=16)
            statsP = ps.tile([16, 2 * W], F32)
            nc.tensor.matmul(statsP[:], zbig[:, 0:16], zbig[:, 0:2 * W],
                             start=True, stop=False, skip_group_check=True)
            for t in range(T):
                lhs = eohb3[:, t, :]
                nc.tensor.matmul(
                    statsP[:, 0:W], lhs, frgr[:, t * W:(t + 1) * W],
                    start=False, stop=False, skip_group_check=True)
                nc.tensor.matmul(
                    statsP[:, W:2 * W], lhs,
                    frgr[:, (T + t) * W:(T + t + 1) * W],
                    start=False, stop=(t == T - 1), skip_group_check=True)

            # ---------------- coefficients: P path first ----------------
            cntS = sb.tile([16, 1], F32)
            nc.vector.tensor_copy(cntS[:], statsP[:, D:D + 1])
            alpha = sb.tile([16, 1], F32)
            nc.vector.tensor_scalar(alpha[:], cntS[:], EPS - 1.0, None,
                                    op0=Alu.add)
            nc.vector.reciprocal(alpha[:], alpha[:])
            beta = sb.tile([16, 1], F32)
            nc.vector.tensor_scalar(beta[:], cntS[:], -1.0, float(N) + EPS,
                                    op0=Alu.mult, op1=Alu.add)
            nc.vector.reciprocal(beta[:], beta[:])
            vmask = sb.tile([16, 1], F32)
            nc.vector.tensor_scalar(vmask[:], cntS[:], 1.5, None, op0=Alu.is_ge)
            # P = alpha*cnt - beta*(n-cnt) == (1-eps)*alpha + eps*beta, so
            # P = alpha to ~1e-6 relative -- skip the explicit formula
            Pstack = sb.tile([16, 128], F32)
            nc.vector.tensor_scalar(Pstack[:], onesw, alpha[:], vmask[:],
                                    op0=Alu.mult, op1=Alu.mult)
            pcoef = ps.tile([128, 512], F32)   # one bank: RT|Pb|Qcg|gb
            nc.tensor.transpose(pcoef[:, 16:32], Pstack[:], idf)
            Pbb = sb.tile([128, 32], BF16)
            nc.vector.tensor_copy(Pbb[:, 0:16], pcoef[:, 16:32])
            nc.vector.tensor_copy(Pbb[:, 16:32], pcoef[:, 16:32])

            # ---------------- loss PSUM bank clears ----------------------
            # One full-partition zero matmul per bank tile clears the
            # has_written bits; every real matmul then uses start=False
            # (overwrite where clear, accumulate where set), so the B/A
            # chains for the four 32-partition groups can land in any
            # order.  The dummy overlaps all later regions -> ordered 1st.
            lossPh = [ps.tile([128, 512], F32, name=f"lossP{h}")
                      for h in range(4)]
            for h in range(4):
                nc.tensor.matmul(lossPh[h][:, 0:16],
                                 zbig[:, 0:128], zbig[:, 0:16],
                                 start=True, stop=False,
                                 skip_group_check=True)

            # ------- loss chain B, g-major (group g waits only on DMA/square
            # chunk g): B = sq_i*P[c] via (F^T)^2 @ P_bcast -----------------
            for g in range(NG):
                for h in range(4):
                    nc.tensor.matmul(
                        lossPh[h][32 * g:32 * g + 32, :],
                        Pbb[:], gts[:, GI * g + 512 * h:GI * g + 512 * (h + 1)],
                        start=False, stop=False, tile_position=(0, 32 * g),
                        skip_group_check=True)



            # ---------------- coefficients: R and Q paths ----------------
            statsS = sb.tile([16, 2 * W], F32)
            nc.vector.tensor_copy(statsS[:], statsP[:])
            SqS = sb.tile([16, 1], F32)
            nc.vector.tensor_reduce(SqS[:], statsS[:, W:W + D], axis=AxX,
                                    op=Alu.add)
            gbP = pcoef[0:16, 40:40 + D + 1]
            nc.tensor.matmul(gbP[:, 0:D], ones16, statsS[:, 0:D],
                             start=True, stop=True)
            nc.tensor.matmul(gbP[:, D:D + 1], ones16, SqS[:],
                             start=True, stop=True)
            # Q first (gates mask round 0), then R (gates the A chain)
            ssd = sb.tile([16, 1], F32)
            nc.vector.tensor_tensor(ssd[:], gbP[:, D:D + 1], SqS[:],
                                    op=Alu.subtract)
            nc.vector.tensor_tensor(ssd[:], ssd[:], beta[:], op=Alu.mult)
            QM = sb.tile([16, 1], F32)
            nc.vector.scalar_tensor_tensor(QM[:], SqS[:], alpha[:], ssd[:],
                                           op0=Alu.mult, op1=Alu.subtract)
            nc.vector.tensor_scalar(QM[:], QM[:], MARGIN, vmask[:],
                                    op0=Alu.add, op1=Alu.mult)
            nc.tensor.matmul(pcoef[:, 32:33], ssel, QM[:], start=True,
                             stop=True)
            Qcg = sb.tile([128, 1], F32)
            nc.vector.tensor_copy(Qcg[:], pcoef[:, 32:33])

            tmpd = sb.tile([16, D], F32)
            nc.vector.tensor_tensor(tmpd[:], gbP[:, 0:D], statsS[:, 0:D],
                                    op=Alu.subtract)
            beta2 = sb.tile([16, 1], F32)
            nc.vector.tensor_scalar(beta2[:], beta[:], 2.0, None, op0=Alu.mult)
            nc.vector.tensor_scalar(tmpd[:], tmpd[:], beta2[:], None,
                                    op0=Alu.mult)
            nalpha2 = sb.tile([16, 1], F32)
            nc.vector.tensor_scalar(nalpha2[:], alpha[:], -2.0, None,
                                    op0=Alu.mult)
            Rv = sb.tile([16, D], F32)
            nc.vector.scalar_tensor_tensor(Rv[:], statsS[:, 0:D], nalpha2[:],
                                           tmpd[:], op0=Alu.mult, op1=Alu.add)
            nc.vector.tensor_scalar(Rv[:], Rv[:], vmask[:], None, op0=Alu.mult)
            nc.tensor.transpose(pcoef[:, 0:16], Rv[:], idf)
            RTb = sb.tile([128, 32], BF16)
            nc.vector.tensor_copy(RTb[:, 0:16], pcoef[:, 0:16])
            nc.vector.tensor_copy(RTb[:, 16:32], pcoef[:, 0:16])

            # ------- loss chain A (f_i . R[c]), h-major, mask round h
            # fires as soon as bank h is complete ------------------------
            accq = sb.tile([128, 8], F32)
            nc.gpsimd.memset(accq[:], 0.0)
            nc.vector.tensor_tensor(accq[0:16, 4:5], cntS[:], vmask[:],
                                    op=Alu.mult)
            for h in range(4):
                for g in range(NG):
                    nc.tensor.matmul(
                        lossPh[h][32 * g:32 * g + 32, :],
                        RTb[:], fts[:, GI * g + 512 * h:GI * g + 512 * (h + 1)],
                        start=False, stop=(g == NG - 1),
                        tile_position=(0, 32 * g), skip_group_check=True)
                rh = sb.tile([128, 512], BF16, tag="rh", bufs=2,
                             name=f"rh_{h}")
                nc.scalar.activation(rh[:], lossPh[h][:], Act.Relu,
                                     bias=Qcg[:])
                rel = sb.tile([128, 512], BF16, tag="rel", bufs=2,
                              name=f"rel_{h}")
                nc.vector.scalar_tensor_tensor(
                    rel[:], rh[:], 0.0, eohcs[:, 512 * h:512 * (h + 1)],
                    op0=Alu.add, op1=Alu.mult,
                    accum_out=accq[:, h:h + 1])

            # ---------------- final reduction ----------------
            nc.tensor.matmul(finP[0:1, 0:8], cstbs[:], accq[:],
                             start=True, stop=True, skip_group_check=True)
            fin = sb.tile([1, 8], F32)
            nc.vector.tensor_copy(fin[:], finP[0:1, 0:8])
            numS = sb.tile([1, 1], F32)
            nc.vector.tensor_reduce(numS[:], fin[:, 0:4], axis=AxX, op=Alu.add)
            den = sb.tile([1, 1], F32)
            nc.vector.tensor_scalar(den[:], fin[:, 4:5], 1.0, None, op0=Alu.max)
            nc.vector.reciprocal(den[:], den[:])
            resS = sb.tile([1, 1], F32)
            nc.vector.tensor_tensor(resS[:], numS[:], den[:], op=Alu.mult)
            nc.sync.dma_start(res, resS[:])

    nc.compile()
    _CACHE["nc"] = nc
    return nc


def _make_in_maps(features, labels):
    feats = np.ascontiguousarray(np.asarray(features, dtype=np.float32))
    lab = np.ascontiguousarray(np.asarray(labels)).astype(np.int64)
    fb = feats.astype(BF)                                   # (8192, 128)

    fr = np.zeros((128, T, W), BF)
    fr[:, :, 0:D] = fb.reshape(T, 128, D).transpose(1, 0, 2)
    fr[:, :, D] = 1.0
    ft = np.ascontiguousarray(fb.T)                         # (128, 8192)

    eoh = (lab[:, None] == np.arange(K)[None, :])           # (8192, 16)
    eohb = np.ascontiguousarray(
        eoh.reshape(T, 128, K).transpose(1, 0, 2).reshape(128, T * K)
    ).astype(BF)
    eohcg = np.zeros((128, GI), BF)
    for g in range(NG):
        eohcg[32 * g:32 * g + K, :] = eoh[g * GI:(g + 1) * GI, :].T

    csta = np.zeros((16, 272), np.float32)
    for g in range(NG):
        for q in range(K):
            csta[q, 32 * g + q] = 1.0
            csta[q, 32 * g + 16 + q] = 1.0
    csta[:, 128:144] = np.eye(16, dtype=np.float32)
    csta[:, 144:272] = 1.0
    cstb = np.ones((128, 1), np.float32)

    one = {
        "frd": np.ascontiguousarray(fr.reshape(128, T * W)),
        "ftd": ft,
        "eohb": eohb,
        "eohc": eohcg,
        "csta": csta,
        "cstb": cstb,
    }
    return [dict(one) for _ in range(NCORES)]


def kernel(features, labels):
    nc = _build()
    in_maps = _make_in_maps(features, labels)
    out = run_bass_kernel_spmd(nc, in_maps, core_ids=list(range(NCORES)))
    return np.float32(out.results[0]["res"][0, 0])


# revision 41
# speedup vs baseline: 1.0123x; 1.0123x over previous
"""Trainium2 Bass kernel for nn_ContrastiveDist (supervised contrastive loss).

Math
----
The (n,n) distance/weight loss collapses to per-class statistics.  With
classes c = 0..15, cnt[c], class feature sums C[c,:], squared-norm sums
SqSum[c], global sums Ftot / SSall:

    alpha[c] = 1/(cnt[c]-1+eps),  beta[c] = 1/(n-cnt[c]+eps)
    P[c]   = alpha*cnt - beta*(n-cnt)
    Q[c]   = alpha*SqSum - beta*(SSall-SqSum)
    R[c,:] = 2*beta*(Ftot-C) - 2*alpha*C
    loss_i = f_i . R[c_i] + sq_i*P[c_i] + Q[c_i] + M
    result = sum(relu(loss_i)*valid_i) / max(sum(valid_i), 1)

Device pipeline (per core, replicated on all 8 cores; collectives in this
dispatch path cost ~70us extra so every core redundantly computes the
full loss):
  1. DMA features bf16 in two layouts on one HWDGE FIFO (row-tiled [F|1]
     first, small leading chunk so stats can start early; then d-major
     F^T).  One-hots ride the SWDGE queue (row-tiled early, class-group
     late).
  2. Squares chunk-wise, alternating ACT/DVE.
  3. Stats: per row tile, two contiguous FD-129 matmuls sharing one
     weight load accumulate eoh^T @ [F|1] and eoh^T @ [F^2|1] into a
     (16,258) PSUM bank. A zero-matmul pre-clears the bank so every real
     matmul runs start=False (start=True clears has_written bits for the
     WHOLE bank, which would corrupt the sibling region).
  4. Loss matrix L[c,i] = F^T.R^T + (F^T)^2.P_bcast into four 1-bank
     PSUM tiles (bank h = columns 512h..512h+512 of the row space), each
     laid out as 4 row-groups x 32 classes (classes duplicated into both
     16-halves to keep all PSUM elements written).
  5. Mask, per 512-column bank as soon as its A-chain lands: ACT computes
     relu(L + (Q+M)_percls) (bias = per-partition AP) in bf16, DVE
     multiplies by the class-group one-hot with a free-axis accumulate;
     final partition reduce via ones-matmul; divide; DMA the scalar out.

valid_i folds into the coefficients (R,P,(Q+M) zeroed for cnt<2).
Validated ~4.7e-5 rel vs fp64.
"""

import numpy as np
import ml_dtypes

import concourse.bacc as bacc
import concourse.tile as tile
import concourse.mybir as mybir
from concourse.bass_utils import run_bass_kernel_spmd

N, D, K, NCORES = 8192, 128, 16, 8
T = N // 128               # 64 row tiles of 128
W = D + 1                  # 129: [F | 1]
NG, GI = 4, 2048           # 4 row-groups of 2048 rows (loss layout)
EPS, MARGIN = 1e-6, 10.0
F32 = mybir.dt.float32
BF16 = mybir.dt.bfloat16
Alu = mybir.AluOpType
Act = mybir.ActivationFunctionType
AxX = mybir.AxisListType.X
BF = ml_dtypes.bfloat16

FRB = [0, 4, 16, 32, 48, 64]   # frd chunk boundaries (row tiles)

_CACHE: dict = {}


def _build():
    if "nc" in _CACHE:
        return _CACHE["nc"]

    nc = bacc.Bacc("TRN2", target_bir_lowering=False, debug=False,
                   num_devices=NCORES)
    frd = nc.dram_tensor("frd", [128, T * W], BF16, kind="ExternalInput").ap()
    ftd = nc.dram_tensor("ftd", [128, N], BF16, kind="ExternalInput").ap()
    eohb = nc.dram_tensor("eohb", [128, T * 16], BF16, kind="ExternalInput").ap()
    eohc = nc.dram_tensor("eohc", [128, GI], BF16, kind="ExternalInput").ap()
    csta = nc.dram_tensor("csta", [16, 272], F32, kind="ExternalInput").ap()
    cstb = nc.dram_tensor("cstb", [128, 1], F32, kind="ExternalInput").ap()
    res = nc.dram_tensor("res", [1, 1], F32, kind="ExternalOutput").ap()

    with tile.TileContext(nc) as tc:
        with (
            tc.tile_pool(name="sb", bufs=1) as sb,
            tc.tile_pool(name="ps", bufs=1, space="PSUM") as ps,
        ):
            # zero weights (PSUM bank clears + HAM pacing) -- memset first
            # so it isn't queued behind the gpsimd DMA dispatches
            zbig = sb.tile([128, 512], BF16)
            nc.gpsimd.memset(zbig[:], 0.0)

            # ---------------- loads ----------------
            cstas = sb.tile([16, 272], F32)
            nc.gpsimd.dma_start(cstas[:], csta)
            cstbs = sb.tile([128, 1], F32)
            nc.gpsimd.dma_start(cstbs[:], cstb)
            ssel = cstas[:, 0:128]      # Qcg scatter matrix
            idf = cstas[:, 128:144]     # identity 16x16
            ones16 = cstas[:, 144:160]  # ones (16,16)
            onesw = cstas[:, 144:272]   # ones (16,128)

            eohbs = sb.tile([128, T * 16], BF16)

            frgr = sb.tile([128, 2 * T * W], BF16)   # [F|1] rows then [F^2|1]
            fts = sb.tile([128, N], BF16)
            gts = sb.tile([128, N], BF16)
            eohcs = sb.tile([128, GI], BF16)
            NCT = 8
            TCH = N // NCT       # 1024 (2048B runs)
            # one HWDGE FIFO for all features: the row one-hot first (it
            # gates the first stats matmul), then frd (small leading chunk
            # so the stats stream starts early), then ftd
            nc.sync.dma_start(eohbs[:], eohb)
            for k in range(len(FRB) - 1):
                nc.sync.dma_start(frgr[:, FRB[k] * W:FRB[k + 1] * W],
                                  frd[:, FRB[k] * W:FRB[k + 1] * W])
            for k in range(NCT):
                nc.sync.dma_start(fts[:, k * TCH:(k + 1) * TCH],
                                  ftd[:, k * TCH:(k + 1) * TCH])
            # class-group one-hot: needed only at mask time, keep it last
            nc.gpsimd.dma_start(eohcs[:], eohc)

            # ------- squares, chunk-wise, alternating ACT/DVE -------
            # (ACT Square has no 16-bit accel ~2ns/elem; DVE TT bf16 runs
            # 2x but pays a DRAIN ~= dur after each op)
            for k in range(len(FRB) - 1):
                src = frgr[:, FRB[k] * W:FRB[k + 1] * W]
                dst = frgr[:, (T + FRB[k]) * W:(T + FRB[k + 1]) * W]
                if k % 2 == 0:
                    nc.scalar.activation(dst, src, Act.Square)
                else:
                    nc.vector.tensor_tensor(dst, src, src, op=Alu.mult)
            for k in range(NCT):
                src = fts[:, k * TCH:(k + 1) * TCH]
                dst = gts[:, k * TCH:(k + 1) * TCH]
                if k % 2 == 0:
                    nc.vector.tensor_tensor(dst, src, src, op=Alu.mult)
                else:
                    nc.scalar.activation(dst, src, Act.Square)

            # ---------------- stats: eoh^T @ [F|1|F^2|1] ----------------
            finP = ps.tile([1, 512], F32)
            eohb3 = eohbs.rearrange("p (t c) -> p t c", c=16)
            statsP = ps.tile([16, 2 * W], F32)
            nc.tensor.matmul(statsP[:], zbig[:, 0:16], zbig[:, 0:2 * W],
                             start=True, stop=False, skip_group_check=True)
            for t in range(T):
                lhs = eohb3[:, t, :]
                nc.tensor.matmul(
                    statsP[:, 0:W], lhs, frgr[:, t * W:(t + 1) * W],
                    start=False, stop=False, skip_group_check=True)
                nc.tensor.matmul(
                    statsP[:, W:2 * W], lhs,
                    frgr[:, (T + t) * W:(T + t + 1) * W],
                    start=False, stop=(t == T - 1), skip_group_check=True)

            # ---------------- coefficients: P path first ----------------
            cntS = sb.tile([16, 1], F32)
            nc.vector.tensor_copy(cntS[:], statsP[:, D:D + 1])
            alpha = sb.tile([16, 1], F32)
            nc.vector.tensor_scalar(alpha[:], cntS[:], EPS - 1.0, None,
                                    op0=Alu.add)
            nc.vector.reciprocal(alpha[:], alpha[:])
            beta = sb.tile([16, 1], F32)
            nc.vector.tensor_scalar(beta[:], cntS[:], -1.0, float(N) + EPS,
                                    op0=Alu.mult, op1=Alu.add)
            nc.vector.reciprocal(beta[:], beta[:])
            vmask = sb.tile([16, 1], F32)
            nc.vector.tensor_scalar(vmask[:], cntS[:], 1.5, None, op0=Alu.is_ge)
            # P = alpha*cnt - beta*(n-cnt) == (1-eps)*alpha + eps*beta, so
            # P = alpha to ~1e-6 relative -- skip the explicit formula
            Pstack = sb.tile([16, 128], F32)
            nc.vector.tensor_scalar(Pstack[:], onesw, alpha[:], vmask[:],
                                    op0=Alu.mult, op1=Alu.mult)
            pcoef = ps.tile([128, 512], F32)   # one bank: RT|Pb|Qcg|gb
            nc.tensor.transpose(pcoef[:, 16:32], Pstack[:], idf)
            Pbb = sb.tile([128, 32], BF16)
            nc.vector.tensor_copy(Pbb[:, 0:16], pcoef[:, 16:32])
            nc.vector.tensor_copy(Pbb[:, 16:32], pcoef[:, 16:32])

            # ---------------- loss PSUM bank clears ----------------------
            # One full-partition zero matmul per bank tile clears the
            # has_written bits; every real matmul then uses start=False
            # (overwrite where clear, accumulate where set), so the B/A
            # chains for the four 32-partition groups can land in any
            # order.  The dummy overlaps all later regions -> ordered 1st.
            lossPh = [ps.tile([128, 512], F32, name=f"lossP{h}")
                      for h in range(4)]
            for h in range(4):
                nc.tensor.matmul(lossPh[h][:, 0:16],
                                 zbig[:, 0:128], zbig[:, 0:16],
                                 start=True, stop=False,
                                 skip_group_check=True)

            # ------- loss chain B, g-major (group g waits only on DMA/square
            # chunk g): B = sq_i*P[c] via (F^T)^2 @ P_bcast -----------------
            for g in range(NG):
                for h in range(4):
                    nc.tensor.matmul(
                        lossPh[h][32 * g:32 * g + 32, :],
                        Pbb[:], gts[:, GI * g + 512 * h:GI * g + 512 * (h + 1)],
                        start=False, stop=False, tile_position=(0, 32 * g),
                        skip_group_check=True)



            # ---------------- coefficients: R and Q paths ----------------
            statsS = sb.tile([16, 2 * W], F32)
            nc.vector.tensor_copy(statsS[:], statsP[:])
            SqS = sb.tile([16, 1], F32)
            nc.vector.tensor_reduce(SqS[:], statsS[:, W:W + D], axis=AxX,
                                    op=Alu.add)
            gbP = pcoef[0:16, 40:40 + D + 1]
            nc.tensor.matmul(gbP[:, 0:D], ones16, statsS[:, 0:D],
                             start=True, stop=True)
            nc.tensor.matmul(gbP[:, D:D + 1], ones16, SqS[:],
                             start=True, stop=True)
            # Q first (gates mask round 0), then R (gates the A chain)
            ssd = sb.tile([16, 1], F32)
            nc.vector.tensor_tensor(ssd[:], gbP[:, D:D + 1], SqS[:],
                                    op=Alu.subtract)
            nc.vector.tensor_tensor(ssd[:], ssd[:], beta[:], op=Alu.mult)
            QM = sb.tile([16, 1], F32)
            nc.vector.scalar_tensor_tensor(QM[:], SqS[:], alpha[:], ssd[:],
                                           op0=Alu.mult, op1=Alu.subtract)
            nc.vector.tensor_scalar(QM[:], QM[:], MARGIN, vmask[:],
                                    op0=Alu.add, op1=Alu.mult)
            nc.tensor.matmul(pcoef[:, 32:33], ssel, QM[:], start=True,
                             stop=True)
            Qcg = sb.tile([128, 1], F32)
            nc.vector.tensor_copy(Qcg[:], pcoef[:, 32:33])

            tmpd = sb.tile([16, D], F32)
            nc.vector.tensor_tensor(tmpd[:], gbP[:, 0:D], statsS[:, 0:D],
                                    op=Alu.subtract)
            beta2 = sb.tile([16, 1], F32)
            nc.vector.tensor_scalar(beta2[:], beta[:], 2.0, None, op0=Alu.mult)
            nc.vector.tensor_scalar(tmpd[:], tmpd[:], beta2[:], None,
                                    op0=Alu.mult)
            nalpha2 = sb.tile([16, 1], F32)
            nc.vector.tensor_scalar(nalpha2[:], alpha[:], -2.0, None,
                                    op0=Alu.mult)
            Rv = sb.tile([16, D], F32)
            nc.vector.scalar_tensor_tensor(Rv[:], statsS[:, 0:D], nalpha2[:],
                                           tmpd[:], op0=Alu.mult, op1=Alu.add)
            nc.vector.tensor_scalar(Rv[:], Rv[:], vmask[:], None, op0=Alu.mult)
            nc.tensor.transpose(pcoef[:, 0:16], Rv[:], idf)
            RTb = sb.tile([128, 32], BF16)
            nc.vector.tensor_copy(RTb[:, 0:16], pcoef[:, 0:16])
            nc.vector.tensor_copy(RTb[:, 16:32], pcoef[:, 0:16])

            # ------- loss chain A (f_i . R[c]), h-major, mask round h
            # fires as soon as bank h is complete ------------------------
            accq = sb.tile([128, 8], F32)
            nc.gpsimd.memset(accq[:], 0.0)
            nc.vector.tensor_tensor(accq[0:16, 4:5], cntS[:], vmask[:],
                                    op=Alu.mult)
            for h in range(4):
                for g in range(NG):
                    nc.tensor.matmul(
                        lossPh[h][32 * g:32 * g + 32, :],
                        RTb[:], fts[:, GI * g + 512 * h:GI * g + 512 * (h + 1)],
                        start=False, stop=(g == NG - 1),
                        tile_position=(0, 32 * g), skip_group_check=True)
                rh = sb.tile([128, 512], BF16, tag="rh", bufs=2,
                             name=f"rh_{h}")
                nc.scalar.activation(rh[:], lossPh[h][:], Act.Relu,
                                     bias=Qcg[:])
                rel = sb.tile([128, 512], BF16, tag="rel", bufs=2,
                              name=f"rel_{h}")
                nc.vector.scalar_tensor_tensor(
                    rel[:], rh[:], 0.0, eohcs[:, 512 * h:512 * (h + 1)],
                    op0=Alu.add, op1=Alu.mult,
                    accum_out=accq[:, h:h + 1])

            # ---------------- final reduction ----------------
            nc.tensor.matmul(finP[0:1, 0:8], cstbs[:], accq[:],
                             start=True, stop=True, skip_group_check=True)
            fin = sb.tile([1, 8], F32)
            nc.vector.tensor_copy(fin[:], finP[0:1, 0:8])
            numS = sb.tile([1, 1], F32)
            nc.vector.tensor_reduce(numS[:], fin[:, 0:4], axis=AxX, op=Alu.add)
            den = sb.tile([1, 1], F32)
            nc.vector.tensor_scalar(den[:], fin[:, 4:5], 1.0, None, op0=Alu.max)
            nc.vector.reciprocal(den[:], den[:])
            resS = sb.tile([1, 1], F32)
            nc.vector.tensor_tensor(resS[:], numS[:], den[:], op=Alu.mult)
            nc.sync.dma_start(res, resS[:])

    nc.compile()
    _CACHE["nc"] = nc
    return nc


def _make_in_maps(features, labels):
    feats = np.ascontiguousarray(np.asarray(features, dtype=np.float32))
    lab = np.ascontiguousarray(np.asarray(labels)).astype(np.int64)
    fb = feats.astype(BF)                                   # (8192, 128)

    fr = np.zeros((128, T, W), BF)
    fr[:, :, 0:D] = fb.reshape(T, 128, D).transpose(1, 0, 2)
    fr[:, :, D] = 1.0
    ft = np.ascontiguousarray(fb.T)                         # (128, 8192)

    eoh = (lab[:, None] == np.arange(K)[None, :])           # (8192, 16)
    eohb = np.ascontiguousarray(
        eoh.reshape(T, 128, K).transpose(1, 0, 2).reshape(128, T * K)
    ).astype(BF)
    eohcg = np.zeros((128, GI), BF)
    for g in range(NG):
        eohcg[32 * g:32 * g + K, :] = eoh[g * GI:(g + 1) * GI, :].T

    csta = np.zeros((16, 272), np.float32)
    for g in range(NG):
        for q in range(K):
            csta[q, 32 * g + q] = 1.0
            csta[q, 32 * g + 16 + q] = 1.0
    csta[:, 128:144] = np.eye(16, dtype=np.float32)
    csta[:, 144:272] = 1.0
    cstb = np.ones((128, 1), np.float32)

    one = {
        "frd": np.ascontiguousarray(fr.reshape(128, T * W)),
        "ftd": ft,
        "eohb": eohb,
        "eohc": eohcg,
        "csta": csta,
        "cstb": cstb,
    }
    return [dict(one) for _ in range(NCORES)]


def kernel(features, labels):
    nc = _build()
    in_maps = _make_in_maps(features, labels)
    out = run_bass_kernel_spmd(nc, in_maps, core_ids=list(range(NCORES)))
    return np.float32(out.results[0]["res"][0, 0])


# revision 42
# speedup vs baseline: 1.0205x; 1.0081x over previous
"""Trainium2 Bass kernel for nn_ContrastiveDist (supervised contrastive loss).

Math
----
The (n,n) distance/weight loss collapses to per-class statistics.  With
classes c = 0..15, cnt[c], class feature sums C[c,:], squared-norm sums
SqSum[c], global sums Ftot / SSall:

    alpha[c] = 1/(cnt[c]-1+eps),  beta[c] = 1/(n-cnt[c]+eps)
    P[c]   = alpha*cnt - beta*(n-cnt)
    Q[c]   = alpha*SqSum - beta*(SSall-SqSum)
    R[c,:] = 2*beta*(Ftot-C) - 2*alpha*C
    loss_i = f_i . R[c_i] + sq_i*P[c_i] + Q[c_i] + M
    result = sum(relu(loss_i)*valid_i) / max(sum(valid_i), 1)

Device pipeline (per core, replicated on all 8 cores; collectives in this
dispatch path cost ~70us extra so every core redundantly computes the
full loss):
  1. DMA features bf16 in two layouts on one HWDGE FIFO (row-tiled [F|1]
     first, small leading chunk so stats can start early; then d-major
     F^T).  One-hots ride the SWDGE queue (row-tiled early, class-group
     late).
  2. Squares chunk-wise, alternating ACT/DVE.
  3. Stats: per row tile, two contiguous FD-129 matmuls sharing one
     weight load accumulate eoh^T @ [F|1] and eoh^T @ [F^2|1] into a
     (16,258) PSUM bank. A zero-matmul pre-clears the bank so every real
     matmul runs start=False (start=True clears has_written bits for the
     WHOLE bank, which would corrupt the sibling region).
  4. Loss matrix L[c,i] = F^T.R^T + (F^T)^2.P_bcast into four 1-bank
     PSUM tiles (bank h = columns 512h..512h+512 of the row space), each
     laid out as 4 row-groups x 32 classes (classes duplicated into both
     16-halves to keep all PSUM elements written).
  5. Mask, per 512-column bank as soon as its A-chain lands: ACT computes
     relu(L + (Q+M)_percls) (bias = per-partition AP) in bf16, DVE
     multiplies by the class-group one-hot with a free-axis accumulate;
     final partition reduce via ones-matmul; divide; DMA the scalar out.

valid_i folds into the coefficients (R,P,(Q+M) zeroed for cnt<2).
Validated ~4.7e-5 rel vs fp64.
"""

import numpy as np
import ml_dtypes

import concourse.bacc as bacc
import concourse.tile as tile
import concourse.mybir as mybir
from concourse.bass_utils import run_bass_kernel_spmd

N, D, K, NCORES = 8192, 128, 16, 8
T = N // 128               # 64 row tiles of 128
W = D + 1                  # 129: [F | 1]
NG, GI = 4, 2048           # 4 row-groups of 2048 rows (loss layout)
EPS, MARGIN = 1e-6, 10.0
F32 = mybir.dt.float32
BF16 = mybir.dt.bfloat16
Alu = mybir.AluOpType
Act = mybir.ActivationFunctionType
AxX = mybir.AxisListType.X
BF = ml_dtypes.bfloat16

FRB = [0, 4, 16, 32, 48, 64]   # frd chunk boundaries (row tiles)

_CACHE: dict = {}


def _build():
    if "nc" in _CACHE:
        return _CACHE["nc"]

    nc = bacc.Bacc("TRN2", target_bir_lowering=False, debug=False,
                   num_devices=NCORES)
    frd = nc.dram_tensor("frd", [128, T * W], BF16, kind="ExternalInput").ap()
    ftd = nc.dram_tensor("ftd", [128, N], BF16, kind="ExternalInput").ap()
    eohb = nc.dram_tensor("eohb", [128, T * 16], BF16, kind="ExternalInput").ap()
    eohc = nc.dram_tensor("eohc", [128, GI], BF16, kind="ExternalInput").ap()
    csta = nc.dram_tensor("csta", [16, 272], F32, kind="ExternalInput").ap()
    cstb = nc.dram_tensor("cstb", [128, 1], F32, kind="ExternalInput").ap()
    res = nc.dram_tensor("res", [1, 1], F32, kind="ExternalOutput").ap()

    with tile.TileContext(nc) as tc:
        with (
            tc.tile_pool(name="sb", bufs=1) as sb,
            tc.tile_pool(name="ps", bufs=1, space="PSUM") as ps,
        ):
            # zero weights (PSUM bank clears + HAM pacing) -- memset first
            # so it isn't queued behind the gpsimd DMA dispatches
            zbig = sb.tile([128, 512], BF16)
            nc.gpsimd.memset(zbig[:], 0.0)

            # ---------------- loads ----------------
            cstas = sb.tile([16, 272], F32)
            nc.gpsimd.dma_start(cstas[:], csta)
            cstbs = sb.tile([128, 1], F32)
            nc.gpsimd.dma_start(cstbs[:], cstb)
            ssel = cstas[:, 0:128]      # Qcg scatter matrix
            idf = cstas[:, 128:144]     # identity 16x16
            ones16 = cstas[:, 144:160]  # ones (16,16)
            onesw = cstas[:, 144:272]   # ones (16,128)

            eohbs = sb.tile([128, T * 16], BF16)

            frgr = sb.tile([128, 2 * T * W], BF16)   # [F|1] rows then [F^2|1]
            fts = sb.tile([128, N], BF16)
            gts = sb.tile([128, N], BF16)
            eohcs = sb.tile([128, GI], BF16)
            NCT = 8
            TCH = N // NCT       # 1024 (2048B runs)
            # one HWDGE FIFO for all features: the row one-hot first (it
            # gates the first stats matmul), then frd (small leading chunk
            # so the stats stream starts early), then ftd
            nc.sync.dma_start(eohbs[:], eohb)
            for k in range(len(FRB) - 1):
                nc.sync.dma_start(frgr[:, FRB[k] * W:FRB[k + 1] * W],
                                  frd[:, FRB[k] * W:FRB[k + 1] * W])
            for k in range(NCT):
                nc.sync.dma_start(fts[:, k * TCH:(k + 1) * TCH],
                                  ftd[:, k * TCH:(k + 1) * TCH])
            # class-group one-hot: needed only at mask time, keep it last
            nc.gpsimd.dma_start(eohcs[:], eohc)

            # ------- squares, chunk-wise, alternating ACT/DVE -------
            # (ACT Square has no 16-bit accel ~2ns/elem; DVE TT bf16 runs
            # 2x but pays a DRAIN ~= dur after each op)
            for k in range(len(FRB) - 1):
                src = frgr[:, FRB[k] * W:FRB[k + 1] * W]
                dst = frgr[:, (T + FRB[k]) * W:(T + FRB[k + 1]) * W]
                if k == 0 or k % 2 == 1:
                    # chunk 0 on DVE: it gates the first stats matmul and
                    # DVE has no ACT-table-load latency in front of it
                    nc.vector.tensor_tensor(dst, src, src, op=Alu.mult)
                else:
                    nc.scalar.activation(dst, src, Act.Square)
            for k in range(NCT):
                src = fts[:, k * TCH:(k + 1) * TCH]
                dst = gts[:, k * TCH:(k + 1) * TCH]
                if k % 2 == 0:
                    nc.vector.tensor_tensor(dst, src, src, op=Alu.mult)
                else:
                    nc.scalar.activation(dst, src, Act.Square)

            # ---------------- stats: eoh^T @ [F|1|F^2|1] ----------------
            finP = ps.tile([1, 512], F32)
            eohb3 = eohbs.rearrange("p (t c) -> p t c", c=16)
            statsP = ps.tile([16, 2 * W], F32)
            nc.tensor.matmul(statsP[:], zbig[:, 0:16], zbig[:, 0:2 * W],
                             start=True, stop=False, skip_group_check=True)
            for t in range(T):
                lhs = eohb3[:, t, :]
                nc.tensor.matmul(
                    statsP[:, 0:W], lhs, frgr[:, t * W:(t + 1) * W],
                    start=False, stop=False, skip_group_check=True)
                nc.tensor.matmul(
                    statsP[:, W:2 * W], lhs,
                    frgr[:, (T + t) * W:(T + t + 1) * W],
                    start=False, stop=(t == T - 1), skip_group_check=True)

            # ---------------- coefficients: P path first ----------------
            cntS = sb.tile([16, 1], F32)
            nc.vector.tensor_copy(cntS[:], statsP[:, D:D + 1])
            alpha = sb.tile([16, 1], F32)
            nc.vector.tensor_scalar(alpha[:], cntS[:], EPS - 1.0, None,
                                    op0=Alu.add)
            nc.vector.reciprocal(alpha[:], alpha[:])
            beta = sb.tile([16, 1], F32)
            nc.vector.tensor_scalar(beta[:], cntS[:], -1.0, float(N) + EPS,
                                    op0=Alu.mult, op1=Alu.add)
            nc.vector.reciprocal(beta[:], beta[:])
            vmask = sb.tile([16, 1], F32)
            nc.vector.tensor_scalar(vmask[:], cntS[:], 1.5, None, op0=Alu.is_ge)
            # P = alpha*cnt - beta*(n-cnt) == (1-eps)*alpha + eps*beta, so
            # P = alpha to ~1e-6 relative -- skip the explicit formula
            Pstack = sb.tile([16, 128], F32)
            nc.vector.tensor_scalar(Pstack[:], onesw, alpha[:], vmask[:],
                                    op0=Alu.mult, op1=Alu.mult)
            pcoef = ps.tile([128, 512], F32)   # one bank: RT|Pb|Qcg|gb
            nc.tensor.transpose(pcoef[:, 16:32], Pstack[:], idf)
            Pbb = sb.tile([128, 32], BF16)
            nc.vector.tensor_copy(Pbb[:, 0:16], pcoef[:, 16:32])
            nc.vector.tensor_copy(Pbb[:, 16:32], pcoef[:, 16:32])

            # ---------------- loss PSUM bank clears ----------------------
            # One full-partition zero matmul per bank tile clears the
            # has_written bits; every real matmul then uses start=False
            # (overwrite where clear, accumulate where set), so the B/A
            # chains for the four 32-partition groups can land in any
            # order.  The dummy overlaps all later regions -> ordered 1st.
            lossPh = [ps.tile([128, 512], F32, name=f"lossP{h}")
                      for h in range(4)]
            for h in range(4):
                nc.tensor.matmul(lossPh[h][:, 0:16],
                                 zbig[:, 0:128], zbig[:, 0:16],
                                 start=True, stop=False,
                                 skip_group_check=True)

            # ------- loss chain B, g-major (group g waits only on DMA/square
            # chunk g): B = sq_i*P[c] via (F^T)^2 @ P_bcast -----------------
            for g in range(NG):
                for h in range(4):
                    nc.tensor.matmul(
                        lossPh[h][32 * g:32 * g + 32, :],
                        Pbb[:], gts[:, GI * g + 512 * h:GI * g + 512 * (h + 1)],
                        start=False, stop=False, tile_position=(0, 32 * g),
                        skip_group_check=True)



            # ---------------- coefficients: R and Q paths ----------------
            statsS = sb.tile([16, 2 * W], F32)
            nc.vector.tensor_copy(statsS[:], statsP[:])
            SqS = sb.tile([16, 1], F32)
            nc.vector.tensor_reduce(SqS[:], statsS[:, W:W + D], axis=AxX,
                                    op=Alu.add)
            gbP = pcoef[0:16, 40:40 + D + 1]
            nc.tensor.matmul(gbP[:, 0:D], ones16, statsS[:, 0:D],
                             start=True, stop=True)
            nc.tensor.matmul(gbP[:, D:D + 1], ones16, SqS[:],
                             start=True, stop=True)
            # Q first (gates mask round 0), then R (gates the A chain)
            ssd = sb.tile([16, 1], F32)
            nc.vector.tensor_tensor(ssd[:], gbP[:, D:D + 1], SqS[:],
                                    op=Alu.subtract)
            nc.vector.tensor_tensor(ssd[:], ssd[:], beta[:], op=Alu.mult)
            QM = sb.tile([16, 1], F32)
            nc.vector.scalar_tensor_tensor(QM[:], SqS[:], alpha[:], ssd[:],
                                           op0=Alu.mult, op1=Alu.subtract)
            nc.vector.tensor_scalar(QM[:], QM[:], MARGIN, vmask[:],
                                    op0=Alu.add, op1=Alu.mult)
            nc.tensor.matmul(pcoef[:, 32:33], ssel, QM[:], start=True,
                             stop=True)
            Qcg = sb.tile([128, 1], F32)
            nc.vector.tensor_copy(Qcg[:], pcoef[:, 32:33])

            tmpd = sb.tile([16, D], F32)
            nc.vector.tensor_tensor(tmpd[:], gbP[:, 0:D], statsS[:, 0:D],
                                    op=Alu.subtract)
            beta2 = sb.tile([16, 1], F32)
            nc.vector.tensor_scalar(beta2[:], beta[:], 2.0, None, op0=Alu.mult)
            nc.vector.tensor_scalar(tmpd[:], tmpd[:], beta2[:], None,
                                    op0=Alu.mult)
            nalpha2 = sb.tile([16, 1], F32)
            nc.vector.tensor_scalar(nalpha2[:], alpha[:], -2.0, None,
                                    op0=Alu.mult)
            Rv = sb.tile([16, D], F32)
            nc.vector.scalar_tensor_tensor(Rv[:], statsS[:, 0:D], nalpha2[:],
                                           tmpd[:], op0=Alu.mult, op1=Alu.add)
            nc.vector.tensor_scalar(Rv[:], Rv[:], vmask[:], None, op0=Alu.mult)
            nc.tensor.transpose(pcoef[:, 0:16], Rv[:], idf)
            RTb = sb.tile([128, 32], BF16)
            nc.vector.tensor_copy(RTb[:, 0:16], pcoef[:, 0:16])
            nc.vector.tensor_copy(RTb[:, 16:32], pcoef[:, 0:16])

            # ------- loss chain A (f_i . R[c]), h-major, mask round h
            # fires as soon as bank h is complete ------------------------
            accq = sb.tile([128, 8], F32)
            nc.gpsimd.memset(accq[:], 0.0)
            nc.vector.tensor_tensor(accq[0:16, 4:5], cntS[:], vmask[:],
                                    op=Alu.mult)
            for h in range(4):
                for g in range(NG):
                    nc.tensor.matmul(
                        lossPh[h][32 * g:32 * g + 32, :],
                        RTb[:], fts[:, GI * g + 512 * h:GI * g + 512 * (h + 1)],
                        start=False, stop=(g == NG - 1),
                        tile_position=(0, 32 * g), skip_group_check=True)
                rh = sb.tile([128, 512], BF16, tag="rh", bufs=2,
                             name=f"rh_{h}")
                nc.scalar.activation(rh[:], lossPh[h][:], Act.Relu,
                                     bias=Qcg[:])
                rel = sb.tile([128, 512], BF16, tag="rel", bufs=2,
                              name=f"rel_{h}")
                nc.vector.scalar_tensor_tensor(
                    rel[:], rh[:], 0.0, eohcs[:, 512 * h:512 * (h + 1)],
                    op0=Alu.add, op1=Alu.mult,
                    accum_out=accq[:, h:h + 1])

            # ---------------- final reduction ----------------
            nc.tensor.matmul(finP[0:1, 0:8], cstbs[:], accq[:],
                             start=True, stop=True, skip_group_check=True)
            fin = sb.tile([1, 8], F32)
            nc.vector.tensor_copy(fin[:], finP[0:1, 0:8])
            numS = sb.tile([1, 1], F32)
            nc.vector.tensor_reduce(numS[:], fin[:, 0:4], axis=AxX, op=Alu.add)
            den = sb.tile([1, 1], F32)
            nc.vector.tensor_scalar(den[:], fin[:, 4:5], 1.0, None, op0=Alu.max)
            nc.vector.reciprocal(den[:], den[:])
            resS = sb.tile([1, 1], F32)
            nc.vector.tensor_tensor(resS[:], numS[:], den[:], op=Alu.mult)
            nc.sync.dma_start(res, resS[:])

    nc.compile()
    _CACHE["nc"] = nc
    return nc


def _make_in_maps(features, labels):
    feats = np.ascontiguousarray(np.asarray(features, dtype=np.float32))
    lab = np.ascontiguousarray(np.asarray(labels)).astype(np.int64)
    fb = feats.astype(BF)                                   # (8192, 128)

    fr = np.zeros((128, T, W), BF)
    fr[:, :, 0:D] = fb.reshape(T, 128, D).transpose(1, 0, 2)
    fr[:, :, D] = 1.0
    ft = np.ascontiguousarray(fb.T)                         # (128, 8192)

    eoh = (lab[:, None] == np.arange(K)[None, :])           # (8192, 16)
    eohb = np.ascontiguousarray(
        eoh.reshape(T, 128, K).transpose(1, 0, 2).reshape(128, T * K)
    ).astype(BF)
    eohcg = np.zeros((128, GI), BF)
    for g in range(NG):
        eohcg[32 * g:32 * g + K, :] = eoh[g * GI:(g + 1) * GI, :].T

    csta = np.zeros((16, 272), np.float32)
    for g in range(NG):
        for q in range(K):
            csta[q, 32 * g + q] = 1.0
            csta[q, 32 * g + 16 + q] = 1.0
    csta[:, 128:144] = np.eye(16, dtype=np.float32)
    csta[:, 144:272] = 1.0
    cstb = np.ones((128, 1), np.float32)

    one = {
        "frd": np.ascontiguousarray(fr.reshape(128, T * W)),
        "ftd": ft,
        "eohb": eohb,
        "eohc": eohcg,
        "csta": csta,
        "cstb": cstb,
    }
    return [dict(one) for _ in range(NCORES)]


def kernel(features, labels):
    nc = _build()
    in_maps = _make_in_maps(features, labels)
    out = run_bass_kernel_spmd(nc, in_maps, core_ids=list(range(NCORES)))
    return np.float32(out.results[0]["res"][0, 0])
